# revision 41
# baseline (speedup 1.0000x reference)
"""Trainium2 Bass kernel for CrossAttention (sparse_attention variant).

Reference computation (shapes hardcoded):
  x [2, 1024, 1024], context [2, 4, 1024, 1024], doc_similarities [2, 4]
  q = x @ Wq, kv = ctx @ Wkv (k|v), dots = q k^T / sqrt(d) + doc_bias,
  attn = softmax(dots over all 4096 doc tokens), out = (attn @ v) @ Wout + bout

Sharding: 8 cores = 2 batches x 4 head-pairs.  Core c: batch c//4, heads
{2*(c%4), 2*(c%4)+1}.  Each core computes a [1024, 1024] partial of the
output projection (its heads' rows of Wout); host sums 4 partials (with a
bf16->f32 upcast and [128, it, e] -> [n, e] unpack) per batch plus bout.

Design (all matmuls bf16 — fp8/DoubleRow fails the 2e-2 accuracy gate;
each e4m3 site alone measures ~2-5e-2, while all-bf16 is ~5e-3):
  - q/k projected transposed (weights stationary, x/ctx moving); v
    projected directly in natural [j, (h, hd)] layout (ctx slices
    stationary, weights moving), so EV needs no PE transposes of V.
  - Per (head, j-tile): two QK matmuls into a [128, 2, 512] PSUM tile
    (bank-aligned halves), one [128, 1024] exp on ScalarE with fused
    scale + per-partition doc bias, writing bf16 attention weights.
  - Softmax denominator is folded into near-free PE matmuls: per i-slice,
    stationary et[:, 128-slice] x moving ones [128, 1] -> dn^T column
    (output free size 1 => ~1 cycle each).  All 16 (head, i-tile) chains
    accumulate into one PSUM bank via first-touch zeroing (only the very
    first instruction carries start=True).
  - The jc-loop fuses DMA, k/v/q projections and attention; head-1
    attention runs two jc behind head 0 so the st PSUM ring never waits
    on a just-issued exp.  Head 0's EV accumulates inline; head 1's et is
    parked in SBUF and its EV fills the PE pipeline at the tail while
    ScalarE drains the last exps.
  - Normalization: reciprocal of dn^T per head, tiny PE transposes of
    each [128, 1] recip column to a [1, 128] row, ones-broadcast matmuls
    to [128, 128], DVE multiplies into ynT (bf16) -- all woven between EV
    matmuls so the PE->DVE->PE round trips stay off the critical path.
  - Output projection accumulates both heads per [i-tile, 512-col] PSUM
    tile, evicts bf16 into a staging buffer (DVE/Act alternating) and
    writes HBM in a few large chunked DMAs.
"""

import numpy as np
from contextlib import ExitStack

import concourse.bass as bass
import concourse.mybir as mybir
import concourse.tile as tile
from concourse import bacc
from concourse import bass_utils
from concourse.masks import make_identity

try:
    import ml_dtypes
    BF16_NP = np.dtype(ml_dtypes.bfloat16)
except ImportError:  # pragma: no cover
    BF16_NP = None

# Problem constants
B, N, M, CN, D = 2, 1024, 4, 1024, 1024
H = 8          # total heads
HPC = 2        # heads per core
NCORES = 8
HD = D // H    # 128
J = M * CN     # 4096
KT = D // 128  # 8 contraction k-tiles
JC = J // 512  # 8 jc chunks of context columns
JT = J // 128  # 32 j-tiles (attention granularity)
IT = N // 128  # 8 i-tiles
SCALE = float(D ** -0.5)

BF = mybir.dt.bfloat16
F32 = mybir.dt.float32

_NC_CACHE = {}
LAST_RESULT = None


def _build_module(reps=1):
    nc = bacc.Bacc(
        "TRN2",
        target_bir_lowering=False,
        debug=False,
        num_devices=NCORES,
    )

    xT = nc.dram_tensor("xT", [128, KT * N], BF, kind="ExternalInput").ap()
    ctxp = nc.dram_tensor("ctxp", [128, JC, KT * 512], BF, kind="ExternalInput").ap()
    wq = nc.dram_tensor("wq", [128, KT * HPC * HD], BF, kind="ExternalInput").ap()
    wk = nc.dram_tensor("wk", [128, KT * HPC * HD], BF, kind="ExternalInput").ap()
    wv = nc.dram_tensor("wv", [128, KT * HPC * HD], BF, kind="ExternalInput").ap()
    wout = nc.dram_tensor("wout", [128, HPC * D], BF, kind="ExternalInput").ap()
    docb = nc.dram_tensor("docb", [128, JT], F32, kind="ExternalInput").ap()
    outp = nc.dram_tensor("outp", [128, IT * D], BF, kind="ExternalOutput").ap()

    EXP = mybir.ActivationFunctionType.Exp

    with tile.TileContext(nc) as tc:
        with ExitStack() as ctx:
          wpool = ctx.enter_context(tc.tile_pool(name="wpool", bufs=1))
          big = ctx.enter_context(tc.tile_pool(name="big", bufs=1))
          stream = ctx.enter_context(tc.tile_pool(name="stream", bufs=3))
          epool = ctx.enter_context(tc.tile_pool(name="epool", bufs=8))
          spool = ctx.enter_context(tc.tile_pool(name="spool", bufs=2))
          pp = ctx.enter_context(tc.tile_pool(name="pp", bufs=1, space="PSUM"))
          for _rep in range(reps):
              # ---- constants ----
              ones_col_f = wpool.tile([128, 1], F32, name="ones_col_f")
              nc.vector.memset(ones_col_f[:, :], 1.0)
              ones_col = wpool.tile([128, 1], BF, name="ones_col")
              nc.vector.tensor_copy(ones_col[:, :], ones_col_f[:, :])
              ones_row_f = wpool.tile([1, 128], F32, name="ones_row_f")
              nc.vector.memset(ones_row_f[:, :], 1.0)
              ones_row = wpool.tile([1, 128], BF, name="ones_row")
              nc.vector.tensor_copy(ones_row[:, :], ones_row_f[:, :])
              ident = wpool.tile([128, 128], F32, name="ident")
              make_identity(nc, ident[:, :])

              docb_sb = wpool.tile([128, JT], F32, name="docb_sb")

              # weights
              wq_sb = wpool.tile([128, KT * HPC * HD], BF, name="wq_sb")
              wk_sb = wpool.tile([128, KT * HPC * HD], BF, name="wk_sb")
              wv_sb = wpool.tile([128, KT * HPC * HD], BF, name="wv_sb")
              wout_sb = wpool.tile([128, HPC * D], BF, name="wout_sb")

              # activations (persistent)
              xT_sb = big.tile([128, KT * N], BF, name="xT_sb")
              qT_sb = big.tile([128, HPC, N], BF, name="qT_sb")
              kT_sb = big.tile([128, HPC, J], BF, name="kT_sb")
              vnat_sb = big.tile([128, JT, HPC, HD], BF, name="vnat_sb")
              et1_sb = big.tile([128, JT, 2, 512], BF, name="et1_sb")  # h1 weights
              ynT_sb = big.tile([128, HPC, N], BF, name="ynT_sb")
              ostage = big.tile([128, IT * D], BF, name="ostage")

              # k weights first: the jc0 k-projection is the first PE work.
              nc.sync.dma_start(out=wk_sb[:, 0:1024], in_=wk[:, 0:1024])
              nc.sync.dma_start(out=wk_sb[:, 1024:2048], in_=wk[:, 1024:2048])

              def emit_qproj(h, ic):
                  qp = pp.tile([128, 512], F32, name="qp", tag="st", bufs=2)
                  for kt in range(KT):
                      nc.tensor.matmul(
                          qp[:, :],
                          lhsT=wq_sb[:, kt * 256 + h * HD:kt * 256 + (h + 1) * HD],
                          rhs=xT_sb[:, ic * 4096 + kt * 512:ic * 4096 + (kt + 1) * 512],
                          start=(kt == 0),
                          stop=(kt == KT - 1),
                      )
                  nc.vector.tensor_copy(
                      qT_sb[:, h, ic * 512:(ic + 1) * 512], qp[:, :]
                  )

              # dn^T accumulator: one PSUM bank, 16 chains (h*8+it), only the
              # very first matmul in the bank carries start=True (first-touch
              # zeroing of the 2KB region covers every chain's first write).
              dn_all = pp.tile([128, 16], F32, name="dn_all", tag="dn", bufs=1)
              y_tiles = {}

              def emit_attention(h, j):
                  st = pp.tile([128, 2, 512], F32, name="st", tag="st", bufs=2)
                  for ic in range(2):
                      nc.tensor.matmul(
                          st[:, ic, :],
                          lhsT=kT_sb[:, h, j * 128:(j + 1) * 128],
                          rhs=qT_sb[:, h, ic * 512:(ic + 1) * 512],
                          start=True,
                          stop=True,
                      )
                  if h == 0:
                      et = epool.tile([128, 2, 512], BF, name="et", tag="et")
                  else:
                      et = et1_sb[:, j, :, :]
                  nc.scalar.activation(
                      et[:, :, :],
                      st[:, :, :],
                      EXP,
                      bias=docb_sb[:, j:j + 1],
                      scale=SCALE,
                  )
                  for it in range(IT):
                      ic, col = it // 4, (it % 4) * 128
                      nc.tensor.matmul(
                          dn_all[:, h * IT + it:h * IT + it + 1],
                          lhsT=et[:, ic, col:col + 128],
                          rhs=ones_col[:, :],
                          start=(h == 0 and j == 0 and it == 0),
                          stop=(j == JT - 1),
                          skip_group_check=True,
                      )
                  if h == 0:
                      if 0 not in y_tiles:
                          y_tiles[0] = pp.tile(
                              [128, 2, 512], F32, name="y0", tag="y", bufs=1
                          )
                      for ic in range(2):
                          nc.tensor.matmul(
                              y_tiles[0][:, ic, :],
                              lhsT=vnat_sb[:, j, 0, :],
                              rhs=et[:, ic, :],
                              start=(j == 0),
                              stop=(j == JT - 1),
                          )

              def emit_vproj(ct, jc, jt2):
                  jt = jc * 4 + jt2
                  vp = pp.tile([128, HPC, HD], F32, name="vp", tag="vp", bufs=1)
                  for kt in range(KT):
                      nc.tensor.matmul(
                          vp[:, :, :],
                          lhsT=ct[:, kt * 512 + jt2 * 128:kt * 512 + (jt2 + 1) * 128],
                          rhs=wv_sb[:, kt * 256:(kt + 1) * 256],
                          start=(kt == 0),
                          stop=(kt == KT - 1),
                      )
                  nc.vector.tensor_copy(vnat_sb[:, jt, :, :], vp[:, :, :])

              def emit_kproj(ct, jc, h):
                  kp = pp.tile([128, 512], F32, name="kp", tag="st", bufs=2)
                  for kt in range(KT):
                      nc.tensor.matmul(
                          kp[:, :],
                          lhsT=wk_sb[:, kt * 256 + h * HD:kt * 256 + (h + 1) * HD],
                          rhs=ct[:, kt * 512:(kt + 1) * 512],
                          start=(kt == 0),
                          stop=(kt == KT - 1),
                      )
                  nc.vector.tensor_copy(
                      kT_sb[:, h, jc * 512:(jc + 1) * 512], kp[:, :]
                  )

              # ---- fused projection + attention jc-loop ----
              # vp has a single PSUM slot; the emission order places >=2048
              # PE cycles between consecutive vp chains.  Head-1 attention is
              # emitted one jc late so the st ring never waits on a
              # just-issued exp at the end of each jc.
              for jc in range(JC):
                  ct = stream.tile([128, KT * 512], BF, name="ct", tag="ct")
                  if jc == 0:
                      nc.sync.dma_start(out=ct[:, 0:1024], in_=ctxp[:, 0, 0:1024])
                      nc.sync.dma_start(out=ct[:, 1024:2048], in_=ctxp[:, 0, 1024:2048])
                      nc.sync.dma_start(out=ct[:, 2048:3072], in_=ctxp[:, 0, 2048:3072])
                      nc.sync.dma_start(out=ct[:, 3072:4096], in_=ctxp[:, 0, 3072:4096])
                      nc.sync.dma_start(out=wq_sb[:, :], in_=wq[:, :])
                      nc.sync.dma_start(out=xT_sb[:, 0:4096], in_=xT[:, 0:4096])
                      nc.sync.dma_start(out=xT_sb[:, 4096:8192], in_=xT[:, 4096:8192])
                      nc.sync.dma_start(out=docb_sb[:, :], in_=docb[:, :])
                      nc.sync.dma_start(out=wv_sb[:, :], in_=wv[:, :])
                      nc.sync.dma_start(out=wout_sb[:, :], in_=wout[:, :])
                      emit_kproj(ct, jc, 0)
                      emit_kproj(ct, jc, 1)
                      emit_qproj(0, 0)
                      emit_qproj(0, 1)
                      emit_vproj(ct, jc, 0)
                      emit_qproj(1, 0)
                      emit_vproj(ct, jc, 1)
                      emit_qproj(1, 1)
                      emit_vproj(ct, jc, 2)
                      emit_attention(0, 0)
                      emit_vproj(ct, jc, 3)
                      for jt2 in range(1, 4):
                          emit_attention(0, jt2)
                  else:
                      nc.sync.dma_start(out=ct[:, :], in_=ctxp[:, jc, :])
                      h1 = ([
                          (lambda jt2=jt2: emit_attention(1, (jc - 2) * 4 + jt2))
                          for jt2 in range(4)
                      ] if jc >= 2 else [lambda: None] * 4)
                      emit_kproj(ct, jc, 0)
                      h1[0]()
                      emit_vproj(ct, jc, 0)
                      emit_kproj(ct, jc, 1)
                      h1[1]()
                      emit_vproj(ct, jc, 1)
                      emit_attention(0, jc * 4 + 0)
                      emit_vproj(ct, jc, 2)
                      emit_attention(0, jc * 4 + 1)
                      h1[2]()
                      emit_vproj(ct, jc, 3)
                      emit_attention(0, jc * 4 + 2)
                      h1[3]()
                      emit_attention(0, jc * 4 + 3)

              # ---- normalization / epilogue, pipelined into the tail ----
              recip_sb = spool.tile([128, 16], F32, name="recip_sb", tag="recip")
              rrows = {}
              rs_h1 = {}

              def emit_rtp(i, on_act=False, tag="st"):
                  rtp = pp.tile([1, 128], F32, name="rtp", tag=tag,
                                bufs=(1 if tag == "vp" else 2))
                  nc.tensor.transpose(
                      rtp[:, :], recip_sb[:, i:i + 1], ident[:, :]
                  )
                  rrow = spool.tile([1, 128], BF, name="rrow", tag="rrow", bufs=16)
                  if on_act:
                      nc.scalar.copy(rrow[:, :], rtp[:, :])
                  else:
                      nc.vector.tensor_copy(rrow[:, :], rtp[:, :])
                  rrows[i] = rrow

              def emit_bcp(i, y0f, tag="st"):
                  h, it = i // IT, i % IT
                  bcp = pp.tile([128, 128], F32, name="bcp", tag=tag,
                                bufs=(1 if tag == "vp" else 2))
                  nc.tensor.matmul(
                      bcp[:, :],
                      lhsT=ones_row[:, :],
                      rhs=rrows.pop(i)[:, :],
                      start=True,
                      stop=True,
                  )
                  if h == 0:
                      ic, col = it // 4, (it % 4) * 128
                      nc.vector.tensor_tensor(
                          ynT_sb[:, 0, it * 128:(it + 1) * 128],
                          y0f[:, ic, col:col + 128],
                          bcp[:, :],
                          op=mybir.AluOpType.mult,
                      )
                  else:
                      rs = spool.tile([128, 128], BF, name="rs", tag="rs", bufs=8)
                      nc.scalar.copy(rs[:, :], bcp[:, :])
                      rs_h1[it] = rs

              # evict y0 to SBUF so its PSUM slot frees for y1 (single "y"
              # ring slot); h0's reciprocal only needs h0's dn columns.
              nc.vector.reciprocal(recip_sb[:, 0:8], dn_all[:, 0:8])
              y0f = spool.tile([128, 2, 512], F32, name="y0f", tag="y0f")
              nc.vector.tensor_copy(y0f[:, 0, :], y_tiles[0][:, 0, :])
              nc.scalar.copy(y0f[:, 1, :], y_tiles[0][:, 1, :])

              y_tiles[1] = pp.tile([128, 2, 512], F32, name="y1", tag="y", bufs=1)

              def emit_ev1(j):
                  for ic in range(2):
                      nc.tensor.matmul(
                          y_tiles[1][:, ic, :],
                          lhsT=vnat_sb[:, j, 1, :],
                          rhs=et1_sb[:, j, ic, :],
                          start=(j == 0),
                          stop=(j == JT - 1),
                      )

              # last head-1 attention tiles with EV-h1 j0..23 as PE filler;
              # h0's 8 epilogue pairs woven in at >=2-EV spacing.
              evj = 0
              for k, nev in enumerate([3, 3, 3, 3, 3, 3, 3, 3]):
                  emit_attention(1, 24 + k)
                  emit_ev1(evj); evj += 1
                  emit_rtp(k, tag="vp")
                  emit_ev1(evj); evj += 1
                  if k >= 1:
                      emit_bcp(k - 1, y0f, tag="vp")
                  for _ in range(nev - 2):
                      emit_ev1(evj); evj += 1
              emit_bcp(7, y0f, tag="vp")

              nc.vector.reciprocal(recip_sb[:, 8:16], dn_all[:, 8:16])
              for j in range(24, JT):
                  emit_ev1(j)
                  emit_rtp(8 + (j - 24), on_act=True)

              # evict y1 to SBUF (epilogue mul may read only one PSUM input)
              y1f = spool.tile([128, 2, 512], F32, name="y1f", tag="y0f")
              nc.vector.tensor_copy(y1f[:, 0, :], y_tiles[1][:, 0, :])
              nc.scalar.copy(y1f[:, 1, :], y_tiles[1][:, 1, :])

              def emit_mul_h1(it):
                  ic, col = it // 4, (it % 4) * 128
                  nc.vector.tensor_tensor(
                      ynT_sb[:, 1, it * 128:(it + 1) * 128],
                      y1f[:, ic, col:col + 128],
                      rs_h1.pop(it)[:, :],
                      op=mybir.AluOpType.mult,
                  )

              # ---- output projection ----
              # ops alternate between the "st" ring and the freed "y" slot
              # (3 slots total) so evictions pipeline behind the matmuls.
              emit_bcp(8, None)
              emit_bcp(9, None)
              emit_mul_h1(0)
              emit_mul_h1(1)
              for it in range(IT):
                  if it < 6:
                      emit_bcp(10 + it, None)
                      emit_mul_h1(it + 2)
                  op = pp.tile(
                      [128, 2, 512], F32, name="op",
                      tag=("st" if it % 2 == 0 else "y"),
                      bufs=(2 if it % 2 == 0 else 1),
                  )
                  for oc in range(2):
                      for h in range(HPC):
                          nc.tensor.matmul(
                              op[:, oc, :],
                              lhsT=ynT_sb[:, h, it * 128:(it + 1) * 128],
                              rhs=wout_sb[:, h * D + oc * 512:h * D + (oc + 1) * 512],
                              start=(h == 0),
                              stop=(h == HPC - 1),
                          )
                  dst = ostage[:, it * D:(it + 1) * D]
                  if it >= IT - 2:
                      nc.vector.tensor_copy(
                          ostage[:, it * D:it * D + 512], op[:, 0, :]
                      )
                      nc.scalar.copy(
                          ostage[:, it * D + 512:(it + 1) * D], op[:, 1, :]
                      )
                  elif it % 2 == 0:
                      nc.vector.tensor_copy(dst, op[:, :, :])
                  else:
                      nc.scalar.copy(dst, op[:, :, :])
                  if it == 3:
                      nc.sync.dma_start(
                          out=outp[:, 0:4 * D], in_=ostage[:, 0:4 * D]
                      )
                  elif it == 5:
                      nc.sync.dma_start(
                          out=outp[:, 4 * D:6 * D], in_=ostage[:, 4 * D:6 * D]
                      )
                  elif it == 6:
                      nc.sync.dma_start(
                          out=outp[:, 6 * D:7 * D], in_=ostage[:, 6 * D:7 * D]
                      )
              nc.sync.dma_start(
                  out=outp[:, 7 * D:7 * D + 512], in_=ostage[:, 7 * D:7 * D + 512]
              )
              nc.sync.dma_start(
                  out=outp[:, 7 * D + 512:8 * D], in_=ostage[:, 7 * D + 512:8 * D]
              )

    nc.compile()
    return nc


def get_nc(reps=1):
    if reps not in _NC_CACHE:
        _NC_CACHE[reps] = _build_module(reps)
    return _NC_CACHE[reps]


def make_in_maps(inputs):
    x = np.asarray(inputs["x"], dtype=np.float32)
    context = np.asarray(inputs["context"], dtype=np.float32)
    doc = np.asarray(inputs["doc_similarities"], dtype=np.float32)
    cmask = np.asarray(inputs["context_mask"])
    Wq = np.asarray(inputs["Wq"], dtype=np.float32)
    Wkv = np.asarray(inputs["Wkv"], dtype=np.float32)
    beta = float(np.asarray(inputs["beta"]))
    Wout = np.asarray(inputs["Wout"], dtype=np.float32)

    def bf(a):
        return np.ascontiguousarray(a).astype(BF16_NP)

    per_batch = []
    for b in range(B):
        # xT packed [128, kt, i]
        # [128, ic, kt, 512]: xTp[p, ic*4096 + kt*512 + i2] = x[i, d]
        xTp = bf(x[b].reshape(2, 512, KT, 128).transpose(3, 0, 2, 1))
        # ctx packed [128, jc, kt, j2]
        cb = context[b].reshape(J, D)  # [j, d]
        ctxp = bf(
            cb.reshape(JC, 512, KT, 128).transpose(3, 0, 2, 1)
        )
        bias = np.repeat(doc[b], CN) * beta
        bias = np.where(cmask[b].reshape(J), bias, -1e30).astype(np.float32)
        docbp = np.ascontiguousarray(bias.reshape(JT, 128).T)  # [128, jt]
        per_batch.append((xTp, ctxp, docbp))

    def pack_kxc(w):
        # [D, C] -> [128, KT, C]
        c = w.shape[1]
        return bf(w.reshape(KT, 128, c).transpose(1, 0, 2))

    in_maps = []
    for c in range(NCORES):
        b = c // 4
        h0 = (c % 4) * HPC
        xTp, ctxp, docbp = per_batch[b]
        wout_c = Wout[h0 * HD:(h0 + HPC) * HD, :]
        in_maps.append({
            "xT": xTp,
            "ctxp": ctxp,
            "wq": pack_kxc(Wq[:, h0 * HD:(h0 + HPC) * HD]),
            "wk": pack_kxc(Wkv[:, h0 * HD:(h0 + HPC) * HD]),
            "wv": pack_kxc(Wkv[:, D + h0 * HD:D + (h0 + HPC) * HD]),
            "wout": bf(wout_c.reshape(HPC, 128, D).transpose(1, 0, 2)),
            "docb": docbp,
        })
    return in_maps


def kernel(**inputs):
    global LAST_RESULT
    nc = get_nc()
    in_maps = make_in_maps(inputs)
    res = bass_utils.run_bass_kernel_spmd(
        nc, in_maps, core_ids=list(range(NCORES))
    )
    LAST_RESULT = res
    out = np.zeros((B, N, D), dtype=np.float32)
    for c in range(NCORES):
        part = np.asarray(res.results[c]["outp"]).astype(np.float32)
        # [128, IT, D] -> [N, D] with row it*128+p
        out[c // 4] += part.reshape(128, IT, D).transpose(1, 0, 2).reshape(N, D)
    out += np.asarray(inputs["bout"], dtype=np.float32)
    return out


# revision 42
# speedup vs baseline: 1.0095x; 1.0095x over previous
"""Trainium2 Bass kernel for CrossAttention (sparse_attention variant).

Reference computation (shapes hardcoded):
  x [2, 1024, 1024], context [2, 4, 1024, 1024], doc_similarities [2, 4]
  q = x @ Wq, kv = ctx @ Wkv (k|v), dots = q k^T / sqrt(d) + doc_bias,
  attn = softmax(dots over all 4096 doc tokens), out = (attn @ v) @ Wout + bout

Sharding: 8 cores = 2 batches x 4 head-pairs.  Core c: batch c//4, heads
{2*(c%4), 2*(c%4)+1}.  Each core computes a [1024, 1024] partial of the
output projection (its heads' rows of Wout); host sums 4 partials (with a
bf16->f32 upcast and [128, it, e] -> [n, e] unpack) per batch plus bout.

Design (all matmuls bf16 — fp8/DoubleRow fails the 2e-2 accuracy gate;
each e4m3 site alone measures ~2-5e-2, while all-bf16 is ~5e-3):
  - q/k projected transposed (weights stationary, x/ctx moving); v
    projected directly in natural [j, (h, hd)] layout (ctx slices
    stationary, weights moving), so EV needs no PE transposes of V.
  - Per (head, j-tile): two QK matmuls into a [128, 2, 512] PSUM tile
    (bank-aligned halves), one [128, 1024] exp on ScalarE with fused
    scale + per-partition doc bias, writing bf16 attention weights.
  - Softmax denominator is folded into near-free PE matmuls: per i-slice,
    stationary et[:, 128-slice] x moving ones [128, 1] -> dn^T column
    (output free size 1 => ~1 cycle each).  All 16 (head, i-tile) chains
    accumulate into one PSUM bank via first-touch zeroing (only the very
    first instruction carries start=True).
  - The jc-loop fuses DMA, k/v/q projections and attention; head-1
    attention runs two jc behind head 0 so the st PSUM ring never waits
    on a just-issued exp.  Head 0's EV accumulates inline; head 1's et is
    parked in SBUF and its EV fills the PE pipeline at the tail while
    ScalarE drains the last exps.
  - Normalization: reciprocal of dn^T per head, tiny PE transposes of
    each [128, 1] recip column to a [1, 128] row, ones-broadcast matmuls
    to [128, 128], DVE multiplies into ynT (bf16) -- all woven between EV
    matmuls so the PE->DVE->PE round trips stay off the critical path.
  - Output projection accumulates both heads per [i-tile, 512-col] PSUM
    tile, evicts bf16 into a staging buffer (DVE/Act alternating) and
    writes HBM in a few large chunked DMAs.
"""

import numpy as np
from contextlib import ExitStack

import concourse.bass as bass
import concourse.mybir as mybir
import concourse.tile as tile
from concourse import bacc
from concourse import bass_utils
from concourse.masks import make_identity

try:
    import ml_dtypes
    BF16_NP = np.dtype(ml_dtypes.bfloat16)
except ImportError:  # pragma: no cover
    BF16_NP = None

# Problem constants
B, N, M, CN, D = 2, 1024, 4, 1024, 1024
H = 8          # total heads
HPC = 2        # heads per core
NCORES = 8
HD = D // H    # 128
J = M * CN     # 4096
KT = D // 128  # 8 contraction k-tiles
JC = J // 512  # 8 jc chunks of context columns
JT = J // 128  # 32 j-tiles (attention granularity)
IT = N // 128  # 8 i-tiles
SCALE = float(D ** -0.5)

BF = mybir.dt.bfloat16
F32 = mybir.dt.float32

_NC_CACHE = {}
LAST_RESULT = None


def _build_module(reps=1):
    nc = bacc.Bacc(
        "TRN2",
        target_bir_lowering=False,
        debug=False,
        num_devices=NCORES,
    )

    xT = nc.dram_tensor("xT", [128, KT * N], BF, kind="ExternalInput").ap()
    ctxp = nc.dram_tensor("ctxp", [128, JC, KT * 512], BF, kind="ExternalInput").ap()
    wq = nc.dram_tensor("wq", [128, KT * HPC * HD], BF, kind="ExternalInput").ap()
    wk = nc.dram_tensor("wk", [128, KT * HPC * HD], BF, kind="ExternalInput").ap()
    wv = nc.dram_tensor("wv", [128, KT * HPC * HD], BF, kind="ExternalInput").ap()
    wout = nc.dram_tensor("wout", [128, HPC * D], BF, kind="ExternalInput").ap()
    docb = nc.dram_tensor("docb", [128, JT], F32, kind="ExternalInput").ap()
    outp = nc.dram_tensor("outp", [128, IT * D], BF, kind="ExternalOutput").ap()

    EXP = mybir.ActivationFunctionType.Exp

    with tile.TileContext(nc) as tc:
        with ExitStack() as ctx:
          wpool = ctx.enter_context(tc.tile_pool(name="wpool", bufs=1))
          big = ctx.enter_context(tc.tile_pool(name="big", bufs=1))
          stream = ctx.enter_context(tc.tile_pool(name="stream", bufs=3))
          epool = ctx.enter_context(tc.tile_pool(name="epool", bufs=8))
          spool = ctx.enter_context(tc.tile_pool(name="spool", bufs=2))
          pp = ctx.enter_context(tc.tile_pool(name="pp", bufs=1, space="PSUM"))
          for _rep in range(reps):
              # ---- constants ----
              ones_col_f = wpool.tile([128, 1], F32, name="ones_col_f")
              nc.vector.memset(ones_col_f[:, :], 1.0)
              ones_col = wpool.tile([128, 1], BF, name="ones_col")
              nc.vector.tensor_copy(ones_col[:, :], ones_col_f[:, :])
              ones_row_f = wpool.tile([1, 128], F32, name="ones_row_f")
              nc.vector.memset(ones_row_f[:, :], 1.0)
              ones_row = wpool.tile([1, 128], BF, name="ones_row")
              nc.vector.tensor_copy(ones_row[:, :], ones_row_f[:, :])
              ident = wpool.tile([128, 128], F32, name="ident")
              make_identity(nc, ident[:, :])

              docb_sb = wpool.tile([128, JT], F32, name="docb_sb")

              # weights
              wq_sb = wpool.tile([128, KT * HPC * HD], BF, name="wq_sb")
              wk_sb = wpool.tile([128, KT * HPC * HD], BF, name="wk_sb")
              wv_sb = wpool.tile([128, KT * HPC * HD], BF, name="wv_sb")
              wout_sb = wpool.tile([128, HPC * D], BF, name="wout_sb")

              # activations (persistent)
              xT_sb = big.tile([128, KT * N], BF, name="xT_sb")
              qT_sb = big.tile([128, HPC, N], BF, name="qT_sb")
              kT_sb = big.tile([128, HPC, J], BF, name="kT_sb")
              vnat_sb = big.tile([128, JT, HPC, HD], BF, name="vnat_sb")
              et1_sb = big.tile([128, JT, 2, 512], BF, name="et1_sb")  # h1 weights
              ynT_sb = big.tile([128, HPC, N], BF, name="ynT_sb")
              ostage = big.tile([128, IT * D], BF, name="ostage")

              # k weights first: the jc0 k-projection is the first PE work.
              nc.sync.dma_start(out=wk_sb[:, 0:1024], in_=wk[:, 0:1024])
              nc.sync.dma_start(out=wk_sb[:, 1024:2048], in_=wk[:, 1024:2048])

              def emit_qproj(h, ic):
                  qp = pp.tile([128, 512], F32, name="qp", tag="st", bufs=2)
                  for kt in range(KT):
                      nc.tensor.matmul(
                          qp[:, :],
                          lhsT=wq_sb[:, kt * 256 + h * HD:kt * 256 + (h + 1) * HD],
                          rhs=xT_sb[:, ic * 4096 + kt * 512:ic * 4096 + (kt + 1) * 512],
                          start=(kt == 0),
                          stop=(kt == KT - 1),
                      )
                  nc.vector.tensor_copy(
                      qT_sb[:, h, ic * 512:(ic + 1) * 512], qp[:, :]
                  )

              # dn^T accumulator: one PSUM bank, 16 chains (h*8+it), only the
              # very first matmul in the bank carries start=True (first-touch
              # zeroing of the 2KB region covers every chain's first write).
              dn_all = pp.tile([128, 16], F32, name="dn_all", tag="dn", bufs=1)
              y_tiles = {}

              def emit_attention(h, j):
                  st = pp.tile([128, 2, 512], F32, name="st", tag="st", bufs=2)
                  for ic in range(2):
                      nc.tensor.matmul(
                          st[:, ic, :],
                          lhsT=kT_sb[:, h, j * 128:(j + 1) * 128],
                          rhs=qT_sb[:, h, ic * 512:(ic + 1) * 512],
                          start=True,
                          stop=True,
                      )
                  if h == 0:
                      et = epool.tile([128, 2, 512], BF, name="et", tag="et")
                  else:
                      et = et1_sb[:, j, :, :]
                  nc.scalar.activation(
                      et[:, :, :],
                      st[:, :, :],
                      EXP,
                      bias=docb_sb[:, j:j + 1],
                      scale=SCALE,
                  )
                  for it in range(IT):
                      ic, col = it // 4, (it % 4) * 128
                      nc.tensor.matmul(
                          dn_all[:, h * IT + it:h * IT + it + 1],
                          lhsT=et[:, ic, col:col + 128],
                          rhs=ones_col[:, :],
                          start=(h == 0 and j == 0 and it == 0),
                          stop=(j == JT - 1),
                          skip_group_check=True,
                      )
                  if h == 0:
                      if 0 not in y_tiles:
                          y_tiles[0] = pp.tile(
                              [128, 2, 512], F32, name="y0", tag="y", bufs=1
                          )
                      for ic in range(2):
                          nc.tensor.matmul(
                              y_tiles[0][:, ic, :],
                              lhsT=vnat_sb[:, j, 0, :],
                              rhs=et[:, ic, :],
                              start=(j == 0),
                              stop=(j == JT - 1),
                          )

              def emit_vproj(ct, jc, jt2):
                  jt = jc * 4 + jt2
                  vp = pp.tile([128, HPC, HD], F32, name="vp", tag="vp", bufs=1)
                  for kt in range(KT):
                      nc.tensor.matmul(
                          vp[:, :, :],
                          lhsT=ct[:, kt * 512 + jt2 * 128:kt * 512 + (jt2 + 1) * 128],
                          rhs=wv_sb[:, kt * 256:(kt + 1) * 256],
                          start=(kt == 0),
                          stop=(kt == KT - 1),
                      )
                  nc.vector.tensor_copy(vnat_sb[:, jt, :, :], vp[:, :, :])

              def emit_kproj(ct, jc, h):
                  kp = pp.tile([128, 512], F32, name="kp", tag="st", bufs=2)
                  for kt in range(KT):
                      nc.tensor.matmul(
                          kp[:, :],
                          lhsT=wk_sb[:, kt * 256 + h * HD:kt * 256 + (h + 1) * HD],
                          rhs=ct[:, kt * 512:(kt + 1) * 512],
                          start=(kt == 0),
                          stop=(kt == KT - 1),
                      )
                  nc.vector.tensor_copy(
                      kT_sb[:, h, jc * 512:(jc + 1) * 512], kp[:, :]
                  )

              # ---- fused projection + attention jc-loop ----
              # vp has a single PSUM slot; the emission order places >=2048
              # PE cycles between consecutive vp chains.  Head-1 attention is
              # emitted one jc late so the st ring never waits on a
              # just-issued exp at the end of each jc.
              for jc in range(JC):
                  ct = stream.tile([128, KT * 512], BF, name="ct", tag="ct")
                  if jc == 0:
                      nc.sync.dma_start(out=ct[:, 0:1024], in_=ctxp[:, 0, 0:1024])
                      nc.sync.dma_start(out=ct[:, 1024:2048], in_=ctxp[:, 0, 1024:2048])
                      nc.sync.dma_start(out=ct[:, 2048:3072], in_=ctxp[:, 0, 2048:3072])
                      nc.sync.dma_start(out=ct[:, 3072:4096], in_=ctxp[:, 0, 3072:4096])
                      nc.sync.dma_start(out=wq_sb[:, :], in_=wq[:, :])
                      nc.sync.dma_start(out=xT_sb[:, 0:4096], in_=xT[:, 0:4096])
                      nc.sync.dma_start(out=xT_sb[:, 4096:8192], in_=xT[:, 4096:8192])
                      nc.sync.dma_start(out=docb_sb[:, :], in_=docb[:, :])
                      nc.sync.dma_start(out=wv_sb[:, :], in_=wv[:, :])
                      nc.sync.dma_start(out=wout_sb[:, :], in_=wout[:, :])
                      emit_kproj(ct, jc, 0)
                      emit_kproj(ct, jc, 1)
                      emit_qproj(0, 0)
                      emit_qproj(0, 1)
                      emit_vproj(ct, jc, 0)
                      emit_qproj(1, 0)
                      emit_vproj(ct, jc, 1)
                      emit_qproj(1, 1)
                      emit_vproj(ct, jc, 2)
                      emit_attention(0, 0)
                      emit_vproj(ct, jc, 3)
                      for jt2 in range(1, 4):
                          emit_attention(0, jt2)
                  else:
                      nc.sync.dma_start(out=ct[:, :], in_=ctxp[:, jc, :])
                      emit_kproj(ct, jc, 0)
                      emit_vproj(ct, jc, 0)
                      emit_kproj(ct, jc, 1)
                      emit_vproj(ct, jc, 1)
                      emit_vproj(ct, jc, 2)
                      emit_vproj(ct, jc, 3)
                  if jc > 0:
                      for jt2 in range(4):
                          emit_attention(0, jc * 4 + jt2)
                  if jc >= 2:
                      for jt2 in range(4):
                          emit_attention(1, (jc - 2) * 4 + jt2)

              # ---- normalization / epilogue, pipelined into the tail ----
              recip_sb = spool.tile([128, 16], F32, name="recip_sb", tag="recip")
              rrows = {}
              rs_h1 = {}

              def emit_rtp(i, on_act=False, tag="st"):
                  rtp = pp.tile([1, 128], F32, name="rtp", tag=tag,
                                bufs=(1 if tag == "vp" else 2))
                  nc.tensor.transpose(
                      rtp[:, :], recip_sb[:, i:i + 1], ident[:, :]
                  )
                  rrow = spool.tile([1, 128], BF, name="rrow", tag="rrow", bufs=16)
                  if on_act:
                      nc.scalar.copy(rrow[:, :], rtp[:, :])
                  else:
                      nc.vector.tensor_copy(rrow[:, :], rtp[:, :])
                  rrows[i] = rrow

              def emit_bcp(i, y0f, tag="st"):
                  h, it = i // IT, i % IT
                  bcp = pp.tile([128, 128], F32, name="bcp", tag=tag,
                                bufs=(1 if tag == "vp" else 2))
                  nc.tensor.matmul(
                      bcp[:, :],
                      lhsT=ones_row[:, :],
                      rhs=rrows.pop(i)[:, :],
                      start=True,
                      stop=True,
                  )
                  if h == 0:
                      ic, col = it // 4, (it % 4) * 128
                      nc.vector.tensor_tensor(
                          ynT_sb[:, 0, it * 128:(it + 1) * 128],
                          y0f[:, ic, col:col + 128],
                          bcp[:, :],
                          op=mybir.AluOpType.mult,
                      )
                  else:
                      rs = spool.tile([128, 128], BF, name="rs", tag="rs", bufs=8)
                      nc.scalar.copy(rs[:, :], bcp[:, :])
                      rs_h1[it] = rs

              # evict y0 to SBUF so its PSUM slot frees for y1 (single "y"
              # ring slot); h0's reciprocal only needs h0's dn columns.
              nc.vector.reciprocal(recip_sb[:, 0:8], dn_all[:, 0:8])
              y0f = spool.tile([128, 2, 512], F32, name="y0f", tag="y0f")
              nc.vector.tensor_copy(y0f[:, 0, :], y_tiles[0][:, 0, :])
              nc.scalar.copy(y0f[:, 1, :], y_tiles[0][:, 1, :])

              y_tiles[1] = pp.tile([128, 2, 512], F32, name="y1", tag="y", bufs=1)

              def emit_ev1(j):
                  for ic in range(2):
                      nc.tensor.matmul(
                          y_tiles[1][:, ic, :],
                          lhsT=vnat_sb[:, j, 1, :],
                          rhs=et1_sb[:, j, ic, :],
                          start=(j == 0),
                          stop=(j == JT - 1),
                      )

              # last head-1 attention tiles with EV-h1 j0..23 as PE filler;
              # h0's 8 epilogue pairs woven in at >=2-EV spacing.
              evj = 0
              for k, nev in enumerate([3, 3, 3, 3, 3, 3, 3, 3]):
                  emit_attention(1, 24 + k)
                  emit_ev1(evj); evj += 1
                  emit_rtp(k, tag="vp")
                  emit_ev1(evj); evj += 1
                  if k >= 1:
                      emit_bcp(k - 1, y0f, tag="vp")
                  for _ in range(nev - 2):
                      emit_ev1(evj); evj += 1
              emit_bcp(7, y0f, tag="vp")

              nc.vector.reciprocal(recip_sb[:, 8:16], dn_all[:, 8:16])
              for j in range(24, JT):
                  emit_ev1(j)
                  emit_rtp(8 + (j - 24), on_act=True)

              # evict y1 to SBUF (epilogue mul may read only one PSUM input)
              y1f = spool.tile([128, 2, 512], F32, name="y1f", tag="y0f")
              nc.vector.tensor_copy(y1f[:, 0, :], y_tiles[1][:, 0, :])
              nc.scalar.copy(y1f[:, 1, :], y_tiles[1][:, 1, :])

              def emit_mul_h1(it):
                  ic, col = it // 4, (it % 4) * 128
                  nc.vector.tensor_tensor(
                      ynT_sb[:, 1, it * 128:(it + 1) * 128],
                      y1f[:, ic, col:col + 128],
                      rs_h1.pop(it)[:, :],
                      op=mybir.AluOpType.mult,
                  )

              # ---- output projection ----
              # ops alternate between the "st" ring and the freed "y" slot
              # (3 slots total) so evictions pipeline behind the matmuls.
              emit_bcp(8, None)
              emit_bcp(9, None)
              emit_mul_h1(0)
              emit_mul_h1(1)
              for it in range(IT):
                  if it < 6:
                      emit_bcp(10 + it, None)
                      emit_mul_h1(it + 2)
                  op = pp.tile(
                      [128, 2, 512], F32, name="op",
                      tag=("st" if it % 2 == 0 else "y"),
                      bufs=(2 if it % 2 == 0 else 1),
                  )
                  for oc in range(2):
                      for h in range(HPC):
                          nc.tensor.matmul(
                              op[:, oc, :],
                              lhsT=ynT_sb[:, h, it * 128:(it + 1) * 128],
                              rhs=wout_sb[:, h * D + oc * 512:h * D + (oc + 1) * 512],
                              start=(h == 0),
                              stop=(h == HPC - 1),
                          )
                  dst = ostage[:, it * D:(it + 1) * D]
                  if it >= IT - 2:
                      nc.vector.tensor_copy(
                          ostage[:, it * D:it * D + 512], op[:, 0, :]
                      )
                      nc.scalar.copy(
                          ostage[:, it * D + 512:(it + 1) * D], op[:, 1, :]
                      )
                  elif it % 2 == 0:
                      nc.vector.tensor_copy(dst, op[:, :, :])
                  else:
                      nc.scalar.copy(dst, op[:, :, :])
                  if it == 3:
                      nc.sync.dma_start(
                          out=outp[:, 0:4 * D], in_=ostage[:, 0:4 * D]
                      )
                  elif it == 5:
                      nc.sync.dma_start(
                          out=outp[:, 4 * D:6 * D], in_=ostage[:, 4 * D:6 * D]
                      )
                  elif it == 6:
                      nc.sync.dma_start(
                          out=outp[:, 6 * D:7 * D], in_=ostage[:, 6 * D:7 * D]
                      )
              nc.sync.dma_start(
                  out=outp[:, 7 * D:7 * D + 512], in_=ostage[:, 7 * D:7 * D + 512]
              )
              nc.sync.dma_start(
                  out=outp[:, 7 * D + 512:8 * D], in_=ostage[:, 7 * D + 512:8 * D]
              )

    nc.compile()
    return nc


def get_nc(reps=1):
    if reps not in _NC_CACHE:
        _NC_CACHE[reps] = _build_module(reps)
    return _NC_CACHE[reps]


def make_in_maps(inputs):
    x = np.asarray(inputs["x"], dtype=np.float32)
    context = np.asarray(inputs["context"], dtype=np.float32)
    doc = np.asarray(inputs["doc_similarities"], dtype=np.float32)
    cmask = np.asarray(inputs["context_mask"])
    Wq = np.asarray(inputs["Wq"], dtype=np.float32)
    Wkv = np.asarray(inputs["Wkv"], dtype=np.float32)
    beta = float(np.asarray(inputs["beta"]))
    Wout = np.asarray(inputs["Wout"], dtype=np.float32)

    def bf(a):
        return np.ascontiguousarray(a).astype(BF16_NP)

    per_batch = []
    for b in range(B):
        # xT packed [128, kt, i]
        # [128, ic, kt, 512]: xTp[p, ic*4096 + kt*512 + i2] = x[i, d]
        xTp = bf(x[b].reshape(2, 512, KT, 128).transpose(3, 0, 2, 1))
        # ctx packed [128, jc, kt, j2]
        cb = context[b].reshape(J, D)  # [j, d]
        ctxp = bf(
            cb.reshape(JC, 512, KT, 128).transpose(3, 0, 2, 1)
        )
        bias = np.repeat(doc[b], CN) * beta
        bias = np.where(cmask[b].reshape(J), bias, -1e30).astype(np.float32)
        docbp = np.ascontiguousarray(bias.reshape(JT, 128).T)  # [128, jt]
        per_batch.append((xTp, ctxp, docbp))

    def pack_kxc(w):
        # [D, C] -> [128, KT, C]
        c = w.shape[1]
        return bf(w.reshape(KT, 128, c).transpose(1, 0, 2))

    in_maps = []
    for c in range(NCORES):
        b = c // 4
        h0 = (c % 4) * HPC
        xTp, ctxp, docbp = per_batch[b]
        wout_c = Wout[h0 * HD:(h0 + HPC) * HD, :]
        in_maps.append({
            "xT": xTp,
            "ctxp": ctxp,
            "wq": pack_kxc(Wq[:, h0 * HD:(h0 + HPC) * HD]),
            "wk": pack_kxc(Wkv[:, h0 * HD:(h0 + HPC) * HD]),
            "wv": pack_kxc(Wkv[:, D + h0 * HD:D + (h0 + HPC) * HD]),
            "wout": bf(wout_c.reshape(HPC, 128, D).transpose(1, 0, 2)),
            "docb": docbp,
        })
    return in_maps


def kernel(**inputs):
    global LAST_RESULT
    nc = get_nc()
    in_maps = make_in_maps(inputs)
    res = bass_utils.run_bass_kernel_spmd(
        nc, in_maps, core_ids=list(range(NCORES))
    )
    LAST_RESULT = res
    out = np.zeros((B, N, D), dtype=np.float32)
    for c in range(NCORES):
        part = np.asarray(res.results[c]["outp"]).astype(np.float32)
        # [128, IT, D] -> [N, D] with row it*128+p
        out[c // 4] += part.reshape(128, IT, D).transpose(1, 0, 2).reshape(N, D)
    out += np.asarray(inputs["bout"], dtype=np.float32)
    return out
